# revision 5
# baseline (speedup 1.0000x reference)
"""Trainium2 Bass kernel for nn_AgentGnn (2-layer CGConv GNN, 128 scenes x 64 agents).

Structure exploited:
- Edges are fully-connected per 64-agent scene (no self loops), so gather/scatter
  becomes dense 64x64 blocks: agg[i] = sum_j sigmoid(F_ij) * softplus(S_ij) - diag.
- Per-edge linear terms factor into per-node terms:
    F_ij = af[i] + bf[j],  af = x_i @ Wf[:D] + c_i @ Wf[2D:] (+bias via ACT),
                           bf = x_j @ Wf[D:2D] - c_j @ Wf[2D:]
- Pairwise sums F[d,(i,j)] are built on TensorE with a constant 0/1 indicator
  matmul against a stacked [af_scene; bf_scene] stationary operand.
- softplus = ln(1+exp(.)) (Exp+Ln share one ACT table set); sigmoid in another
  set, so sigmoid passes are batched per SUB-scene group to limit table loads.
- BatchNorm stats are global over all 8192 nodes -> tiny [128,2] AllReduce/layer.
- Sharding: 16 scenes (1024 nodes) per core, pure data parallel otherwise.
"""

import numpy as np

N_SAMPLES = 128
AGENTS = 64
D = 128
EDIM = 2
N = N_SAMPLES * AGENTS
EPS = 1e-5

N_CORES = 8
SCENES_PC = N_SAMPLES // N_CORES      # 16 scenes per core
NODES_PC = SCENES_PC * AGENTS         # 1024 nodes per core
PAIR = AGENTS * AGENTS                # 4096 pairwise cols per scene
CHUNK = 2048                          # pairwise chunk (32 i x 64 j)
N_CHUNKS = PAIR // CHUNK              # 2
I_PER_CHUNK = CHUNK // AGENTS         # 32
SUB = 8                               # scenes per sigmoid sub-batch

_CACHE: dict = {}


def _build_indicator() -> np.ndarray:
    ind = np.zeros((128, PAIR), np.float32)
    for i in range(AGENTS):
        ind[i, i * AGENTS:(i + 1) * AGENTS] = 1.0
    for j in range(AGENTS):
        ind[64 + j, j::AGENTS] = 1.0
    return ind


def _expected_edges():
    a = np.arange(AGENTS)
    rows = np.repeat(a, AGENTS)
    cols = np.tile(a, AGENTS)
    mask = rows != cols
    rows, cols = rows[mask], cols[mask]
    offsets = (np.arange(N_SAMPLES) * AGENTS)[:, None]
    src = (rows[None, :] + offsets).ravel().astype(np.int32)
    dst = (cols[None, :] + offsets).ravel().astype(np.int32)
    return src, dst


def _numpy_fallback(gnn_in, centers, edge_src, edge_dst, ws):
    # generic (slow) reference path, used only if edges don't match the
    # expected block-diagonal fully-connected pattern
    def sigmoid(x):
        return 1.0 / (1.0 + np.exp(-x))

    def softplus(x):
        return np.logaddexp(0.0, x)

    x = gnn_in.astype(np.float64)
    e = (centers[edge_dst] - centers[edge_src]).astype(np.float64)
    for li in (1, 2):
        Wf, bf, Ws, bs, gamma, beta = (ws[f"Wf{li}"], ws[f"bf{li}"], ws[f"Ws{li}"],
                                       ws[f"bs{li}"], ws[f"gamma{li}"], ws[f"beta{li}"])
        z = np.concatenate([x[edge_dst], x[edge_src], e], axis=-1)
        m = sigmoid(z @ Wf + bf) * softplus(z @ Ws + bs)
        agg = np.zeros((N, D))
        np.add.at(agg, edge_dst, m)
        mu = agg.mean(axis=0)
        var = agg.var(axis=0)
        agg = (agg - mu) / np.sqrt(var + EPS) * gamma + beta
        x = np.maximum(agg + x, 0.0)
    return x.astype(np.float32)


def _build_nc(use_collectives=True):
    import concourse.bacc as bacc
    import concourse.mybir as mybir
    import concourse.tile as tile
    from concourse.tile_rust import add_dep_helper

    f32 = mybir.dt.float32
    f16 = mybir.dt.float16
    AF = mybir.ActivationFunctionType
    OP = mybir.AluOpType

    nc = bacc.Bacc("TRN2", target_bir_lowering=False, debug=False,
                   num_devices=N_CORES if use_collectives else 1)

    # ---- I/O ----
    xt_in = nc.dram_tensor("xt", [D, NODES_PC], f32, kind="ExternalInput")
    ct_in = nc.dram_tensor("ct", [EDIM, NODES_PC], f32, kind="ExternalInput")
    ind_in = nc.dram_tensor("ind", [128, PAIR], f32, kind="ExternalInput")
    win = {}
    for li in (1, 2):
        for nm in ("wfd", "wfs", "wsd", "wss"):
            win[f"{nm}{li}"] = nc.dram_tensor(f"{nm}{li}", [D, D], f32, kind="ExternalInput")
        for nm in ("wfe", "wfen", "wse", "wsen"):
            win[f"{nm}{li}"] = nc.dram_tensor(f"{nm}{li}", [EDIM, D], f32, kind="ExternalInput")
        for nm in ("bf", "bs", "ga", "be"):
            win[f"{nm}{li}"] = nc.dram_tensor(f"{nm}{li}", [D, 1], f32, kind="ExternalInput")
    out_t = nc.dram_tensor("out_t", [D, NODES_PC], f32, kind="ExternalOutput")

    acts = []  # ACT instructions in intended engine order

    def act(*args, **kwargs):
        inst = nc.scalar.activation(*args, **kwargs)
        acts.append(inst)
        return inst

    with tile.TileContext(nc) as tc:
        with (
            tc.tile_pool(name="cst", bufs=1) as cst,
            tc.tile_pool(name="wrk", bufs=1) as wrk,
            tc.tile_pool(name="chk", bufs=2) as chk,
            tc.tile_pool(name="ps", bufs=2, space="PSUM") as ps,
            tc.tile_pool(name="dram", bufs=1, space="DRAM") as dram,
        ):
            # ---- load constants ----
            xt = cst.tile([D, NODES_PC], f32)
            ct = cst.tile([EDIM, NODES_PC], f32)
            ind = cst.tile([128, PAIR], f32)
            nc.sync.dma_start(xt[:], xt_in.ap())
            nc.sync.dma_start(ct[:], ct_in.ap())
            nc.sync.dma_start(ind[:], ind_in.ap())
            wt = {}
            for k, h in win.items():
                t = cst.tile(list(h.shape), f32, name=f"t_{k}", tag=f"t_{k}")
                nc.sync.dma_start(t[:], h.ap())
                wt[k] = t

            x_cur = xt
            for li in (1, 2):
                wfd, wfs = wt[f"wfd{li}"], wt[f"wfs{li}"]
                wsd, wss = wt[f"wsd{li}"], wt[f"wss{li}"]
                wfe, wfen = wt[f"wfe{li}"], wt[f"wfen{li}"]
                wse, wsen = wt[f"wse{li}"], wt[f"wsen{li}"]
                bf, bs = wt[f"bf{li}"], wt[f"bs{li}"]
                ga, be = wt[f"ga{li}"], wt[f"be{li}"]

                # ---- node phase: stacked AB = [a_scene(0:64); b_scene(64:128)] ----
                abf, abs_ = [], []
                for s in range(SCENES_PC):
                    xs = x_cur[:, s * AGENTS:(s + 1) * AGENTS]
                    cs = ct[:, s * AGENTS:(s + 1) * AGENTS]
                    for path, (wd, wsrc, we, wen, dst_list) in {
                        "f": (wfd, wfs, wfe, wfen, abf),
                        "s": (wsd, wss, wse, wsen, abs_),
                    }.items():
                        pab = ps.tile([128, D], f32, name=f"pab{path}{s}", tag="pp")
                        nc.tensor.matmul(pab[0:64, :], lhsT=xs, rhs=wd[:], start=True, stop=False)
                        nc.tensor.matmul(pab[0:64, :], lhsT=cs, rhs=we[:], start=False, stop=True)
                        nc.tensor.matmul(pab[64:128, :], lhsT=xs, rhs=wsrc[:], start=True, stop=False)
                        nc.tensor.matmul(pab[64:128, :], lhsT=cs, rhs=wen[:], start=False, stop=True)
                        abt = wrk.tile([128, D], f32, name=f"ab{path}{s}_{li}",
                                       tag=f"ab{path}{s}")
                        nc.vector.tensor_copy(abt[:], pab[:])
                        dst_list.append(abt)

                agg = wrk.tile([D, NODES_PC], f32, name=f"agg{li}", tag="agg")

                # ---- pairwise phases, in sigmoid sub-batches ----
                for b0 in range(0, SCENES_PC, SUB):
                    batch = range(b0, min(b0 + SUB, SCENES_PC))
                    sgs = {}
                    # phase A: sigmoid
                    for s in batch:
                        sg = wrk.tile([D, PAIR], f16, name=f"sg{s % SUB}",
                                      tag=f"sg{s % SUB}")
                        sgs[s] = sg
                        for c in range(N_CHUNKS):
                            pf = ps.tile([D, CHUNK], f32, name=f"pf{s}_{c}", tag="pp")
                            for k in range(CHUNK // 512):
                                col = c * CHUNK + k * 512
                                nc.tensor.matmul(pf[:, k * 512:(k + 1) * 512],
                                                 lhsT=abf[s][:],
                                                 rhs=ind[:, col:col + 512],
                                                 start=True, stop=True)
                            act(sg[:, c * CHUNK:(c + 1) * CHUNK], pf[:],
                                AF.Sigmoid, bias=bf[:, 0:1], scale=1.0)
                    # phase B: exp/ln + mul + segmented reduce + diag subtract
                    for s in batch:
                        for c in range(N_CHUNKS):
                            pspair = ps.tile([D, CHUNK], f32, name=f"psp{s}_{c}", tag="pp")
                            for k in range(CHUNK // 512):
                                col = c * CHUNK + k * 512
                                nc.tensor.matmul(pspair[:, k * 512:(k + 1) * 512],
                                                 lhsT=abs_[s][:],
                                                 rhs=ind[:, col:col + 512],
                                                 start=True, stop=True)
                            act(pspair[:], pspair[:], AF.Exp, bias=bs[:, 0:1], scale=1.0)
                            pch = chk.tile([D, CHUNK], f16, name="pch", tag="pch")
                            act(pch[:], pspair[:], AF.Ln, bias=1.0, scale=1.0)
                            uch = chk.tile([D, CHUNK], f16, name="uch", tag="uch")
                            sgsl = sgs[s][:, c * CHUNK:(c + 1) * CHUNK]
                            nc.vector.tensor_tensor(uch[:], sgsl, pch[:], OP.mult)
                            red = chk.tile([D, I_PER_CHUNK], f32, name="red", tag="red")
                            nc.vector.tensor_reduce(
                                red[:], uch.rearrange("p (i j) -> p i j", j=AGENTS),
                                axis=mybir.AxisListType.X, op=OP.add)
                            dg = chk.tile([D, I_PER_CHUNK], f32, name="dg", tag="dg")
                            base = c * I_PER_CHUNK
                            nc.vector.tensor_copy(
                                dg[:], uch[:, base:base + 31 * 65 + 1:65])
                            off = s * AGENTS + c * I_PER_CHUNK
                            nc.vector.tensor_tensor(
                                agg[:, off:off + I_PER_CHUNK], red[:], dg[:],
                                OP.subtract)

                # ---- BN stats (global over all 8192 nodes) ----
                sq = wrk.tile([D, NODES_PC], f32, name="sq", tag="sq")
                nc.vector.tensor_tensor(sq[:], agg[:], agg[:], OP.mult)
                stats = wrk.tile([D, 2], f32, name=f"stats{li}", tag="stats")
                nc.vector.tensor_reduce(stats[:, 0:1], agg[:],
                                        axis=mybir.AxisListType.X, op=OP.add)
                nc.vector.tensor_reduce(stats[:, 1:2], sq[:],
                                        axis=mybir.AxisListType.X, op=OP.add)
                cc_in = dram.tile([D, 2], f32, name=f"ccin{li}", tag=f"ccin{li}")
                cc_out = dram.tile([D, 2], f32, name=f"ccout{li}", tag=f"ccout{li}",
                                   addr_space="Shared")
                nc.sync.dma_start(cc_in[:], stats[:])
                if use_collectives:
                    nc.gpsimd.collective_compute(
                        "AllReduce", OP.add,
                        replica_groups=[list(range(N_CORES))],
                        ins=[cc_in.opt()], outs=[cc_out.opt()])
                else:
                    nc.sync.dma_start(cc_out[:], cc_in[:])
                stot = wrk.tile([D, 2], f32, name=f"stot{li}", tag="stot")
                nc.sync.dma_start(stot[:], cc_out[:])

                # mu, var, rstd = exp(-0.5*ln(var+eps)); A = gamma*rstd; B = beta-mu*A
                mu = wrk.tile([D, 1], f32, name="mu", tag="mu")
                ex2 = wrk.tile([D, 1], f32, name="ex2", tag="ex2")
                nc.vector.tensor_scalar_mul(mu[:], stot[:, 0:1], 1.0 / N)
                nc.vector.tensor_scalar_mul(ex2[:], stot[:, 1:2], 1.0 / N)
                var = wrk.tile([D, 1], f32, name="var", tag="var")
                nc.vector.tensor_tensor(var[:], mu[:], mu[:], OP.mult)
                nc.vector.tensor_tensor(var[:], ex2[:], var[:], OP.subtract)
                rstd = wrk.tile([D, 1], f32, name="rstd", tag="rstd")
                nc.vector.tensor_scalar_add(var[:], var[:], EPS)
                act(rstd[:], var[:], AF.Ln, bias=0.0, scale=1.0)
                act(rstd[:], rstd[:], AF.Exp, bias=0.0, scale=-0.5)
                A = wrk.tile([D, 1], f32, name="A", tag="A")
                Bt = wrk.tile([D, 1], f32, name="Bt", tag="Bt")
                nc.vector.tensor_tensor(A[:], ga[:], rstd[:], OP.mult)
                nc.vector.tensor_tensor(Bt[:], mu[:], A[:], OP.mult)
                nc.vector.tensor_tensor(Bt[:], be[:], Bt[:], OP.subtract)

                # x_next = relu(agg*A + B + x_cur)
                xn = wrk.tile([D, NODES_PC], f32, name=f"x{li}", tag=f"x{li}")
                nc.vector.tensor_scalar(xn[:], agg[:], A[:, 0:1], Bt[:, 0:1],
                                        OP.mult, OP.add)
                nc.vector.tensor_tensor(xn[:], xn[:], x_cur[:], OP.add)
                nc.vector.tensor_scalar_max(xn[:], xn[:], 0.0)
                x_cur = xn

            nc.sync.dma_start(out_t.ap(), x_cur[:])

        # enforce ACT program order so table loads stay batched
        for a, b in zip(acts, acts[1:]):
            add_dep_helper(b.ins, a.ins, sync=False,
                           reason="ACT table-set batching order")

    nc.compile()
    return nc


def _get_nc():
    if "nc" not in _CACHE:
        _CACHE["nc"] = _build_nc()
    return _CACHE["nc"]


def kernel(**inputs) -> np.ndarray:
    gnn_in = np.ascontiguousarray(np.asarray(inputs["gnn_in"], dtype=np.float32))
    centers = np.ascontiguousarray(np.asarray(inputs["centers"], dtype=np.float32))
    edge_src = np.asarray(inputs["edge_src"], dtype=np.int32)
    edge_dst = np.asarray(inputs["edge_dst"], dtype=np.int32)

    exp_src, exp_dst = _expected_edges()
    if not (np.array_equal(edge_src, exp_src) and np.array_equal(edge_dst, exp_dst)):
        return _numpy_fallback(
            gnn_in, centers, edge_src, edge_dst,
            {k: np.asarray(v, np.float32) for k, v in inputs.items()
             if k not in ("gnn_in", "centers", "edge_src", "edge_dst")})

    from concourse import bass_utils

    common = {"ind": _build_indicator()}
    for li in (1, 2):
        Wf = np.asarray(inputs[f"Wf{li}"], np.float32)
        Ws = np.asarray(inputs[f"Ws{li}"], np.float32)
        common[f"wfd{li}"] = np.ascontiguousarray(Wf[0:D])
        common[f"wfs{li}"] = np.ascontiguousarray(Wf[D:2 * D])
        common[f"wfe{li}"] = np.ascontiguousarray(Wf[2 * D:])
        common[f"wfen{li}"] = np.ascontiguousarray(-Wf[2 * D:])
        common[f"wsd{li}"] = np.ascontiguousarray(Ws[0:D])
        common[f"wss{li}"] = np.ascontiguousarray(Ws[D:2 * D])
        common[f"wse{li}"] = np.ascontiguousarray(Ws[2 * D:])
        common[f"wsen{li}"] = np.ascontiguousarray(-Ws[2 * D:])
        common[f"bf{li}"] = np.asarray(inputs[f"bf{li}"], np.float32).reshape(D, 1)
        common[f"bs{li}"] = np.asarray(inputs[f"bs{li}"], np.float32).reshape(D, 1)
        common[f"ga{li}"] = np.asarray(inputs[f"gamma{li}"], np.float32).reshape(D, 1)
        common[f"be{li}"] = np.asarray(inputs[f"beta{li}"], np.float32).reshape(D, 1)

    in_maps = []
    for c in range(N_CORES):
        sl = slice(c * NODES_PC, (c + 1) * NODES_PC)
        m = dict(common)
        m["xt"] = np.ascontiguousarray(gnn_in[sl].T)
        m["ct"] = np.ascontiguousarray(centers[sl].T)
        in_maps.append(m)

    nc = _get_nc()
    res = bass_utils.run_bass_kernel_spmd(nc, in_maps, core_ids=list(range(N_CORES)))
    out = np.concatenate([r["out_t"] for r in res.results], axis=1)  # [D, N]
    return np.ascontiguousarray(out.T)
